# revision 7
# baseline (speedup 1.0000x reference)
"""Trainium2 Bass kernel for nn_GroupingModule (B=8, N=2048, D=512, H=1024).

Per-core = one batch element (pure data parallel over B across 8 cores).
Pipeline per core:
  conv1d(k=3) as 12 accumulated matmuls per [Hchunk=128, tok=512] PSUM tile
  (x pre-transposed on host to [D, N] layout, zero-padded columns),
  relu(+bias) on ACT -> h, h^2 on DVE,
  LayerNorm+proj folded algebraically: probs[t,j] = s[t]*(q_j[t] - mu[t]*G_j) + C_j
  where q_j = sum_h h*g*pw_j, mu/s from S1=sum h, S2=sum h^2 (ones-matmul reductions),
  z[t] = (p1<=p0) with t=0 forced 0, t=N-1 forced 1,
  r = exclusive prefix sum of z (tensor_tensor_scan),
  grouping_matrix[i,c] = (r[c] == i) via broadcast-matmul + is_equal compare.
"""
import numpy as np
from contextlib import ExitStack

B, N, D, H = 8, 2048, 512, 1024
NCORES = 8
TPP = N // 128  # tokens per partition in token-major reshape (16)
NP2 = N + 2     # padded columns of x^T

FP16_CONV = False  # hi/lo fp16 split for the conv matmuls (3 MMs instead of 1 fp32)

_CACHE = {}


def _patch_tile_tail_drain():
    """This walrus build caps non-EventSemaphore instructions at 1 sem wait.
    TileContext's tail drain accumulates one wait per live processor; split the
    excess into standalone SP nops (program order on SP keeps it sound)."""
    import concourse.tile as tile
    from concourse import mybir
    from concourse.vector_clock import ScopedClock

    if getattr(tile.TileContext, "_drain_patched", False):
        return

    def _drain_and_barrier(self, tick_clock, wait_clock):
        drain_inst = self.nc.sync.drain()
        wait_clock.add_sem_waits(
            drain_inst.ins, ScopedClock({None: tick_clock.global_clock})
        )
        si = drain_inst.ins.sync_info
        if si is not None and si.on_wait and len(si.on_wait) > 1:
            waits = list(si.on_wait)
            drain_inst.ins.sync_info = mybir.SyncInfo(
                on_wait=waits[:1], on_update=list(si.on_update or [])
            )
            for w in waits[1:]:
                nop = self.nc.sync.nop()
                nop.ins.sync_info = mybir.SyncInfo(on_wait=[w], on_update=[])
        self.nc.all_engine_barrier()
        assert self.sems is not None
        popped = self.nc._tile_sem_poison_stack.pop()
        assert popped is self._sem_poison
        self.nc.clear_and_free_semaphores(list(self.sems.allocated().values()))
        self.nc.all_engine_barrier()

    tile.TileContext._drain_and_barrier = _drain_and_barrier
    tile.TileContext._drain_patched = True


def _build_program():
    import concourse.bass as bass
    import concourse.tile as tile
    from concourse import mybir
    import bass_rust as _bass_rust

    _patch_tile_tail_drain()
    f32 = mybir.dt.float32
    f16 = mybir.dt.float16
    nc = bass.Bass()

    # ---- I/O ----
    ins = {}
    if FP16_CONV:
        ins["xth"] = nc.dram_tensor("xth", [128, 4 * NP2], f16, kind="ExternalInput")
        ins["xtl"] = nc.dram_tensor("xtl", [128, 4 * NP2], f16, kind="ExternalInput")
        ins["wth"] = nc.dram_tensor("wth", [128, 12 * H], f16, kind="ExternalInput")
        ins["wtl"] = nc.dram_tensor("wtl", [128, 12 * H], f16, kind="ExternalInput")
    else:
        ins["xt"] = nc.dram_tensor("xt", [128, 4 * NP2], f32, kind="ExternalInput")
        ins["wt"] = nc.dram_tensor("wt", [128, 12 * H], f32, kind="ExternalInput")
    ins["aug"] = nc.dram_tensor("aug", [128, 32], f32, kind="ExternalInput")
    ins["bias"] = nc.dram_tensor("bias", [128, 8], f32, kind="ExternalInput")
    ins["epi"] = nc.dram_tensor("epi", [128, 4], f32, kind="ExternalInput")
    ins["rowbase"] = nc.dram_tensor("rowbase", [128, 16], f32, kind="ExternalInput")
    ins["ones1"] = nc.dram_tensor("ones1", [1, 128], f32, kind="ExternalInput")
    probs_ext = nc.dram_tensor("probs", [2, 128, TPP], f32, kind="ExternalOutput")
    gm_ext = nc.dram_tensor("gm", [N, N], f32, kind="ExternalOutput")

    AL = mybir.AluOpType
    AF = mybir.ActivationFunctionType

    with tile.TileContext(nc) as tc, ExitStack() as ctx:
        const = ctx.enter_context(tc.tile_pool(name="const", bufs=1))
        work = ctx.enter_context(tc.tile_pool(name="work", bufs=3))
        small = ctx.enter_context(tc.tile_pool(name="small", bufs=1))
        ohp = ctx.enter_context(tc.tile_pool(name="ohp", bufs=4))
        psc = ctx.enter_context(tc.tile_pool(name="psc", bufs=2, space="PSUM"))
        pst = ctx.enter_context(tc.tile_pool(name="pst", bufs=2, space="PSUM"))
        psr = ctx.enter_context(tc.tile_pool(name="psr", bufs=2, space="PSUM"))

        # ---- load constants ----
        if FP16_CONV:
            xth_t = const.tile([128, 4 * NP2], f16, tag="xth")
            xtl_t = const.tile([128, 4 * NP2], f16, tag="xtl")
            wth_t = const.tile([128, 12 * H], f16, tag="wth")
            wtl_t = const.tile([128, 12 * H], f16, tag="wtl")
            nc.sync.dma_start(xth_t[:], ins["xth"][:])
            nc.sync.dma_start(xtl_t[:], ins["xtl"][:])
            nc.sync.dma_start(wth_t[:], ins["wth"][:])
            nc.sync.dma_start(wtl_t[:], ins["wtl"][:])
        else:
            xt_t = const.tile([128, 4 * NP2], f32, tag="xt")
            wt_t = const.tile([128, 12 * H], f32, tag="wt")
            nc.sync.dma_start(xt_t[:], ins["xt"][:])
            nc.sync.dma_start(wt_t[:], ins["wt"][:])
        aug_t = const.tile([128, 32], f32, tag="aug")
        bias_t = const.tile([128, 8], f32, tag="bias")
        epi_t = const.tile([128, 4], f32, tag="epi")
        rowbase_t = const.tile([128, 16], f32, tag="rowbase")
        ones1_t = const.tile([1, 128], f32, tag="ones1")
        for name, t in [("aug", aug_t), ("bias", bias_t), ("epi", epi_t),
                        ("rowbase", rowbase_t), ("ones1", ones1_t)]:
            nc.sync.dma_start(t[:], ins[name][:])

        # ---- persistent small tiles ----
        q0_t = small.tile([128, TPP], f32, tag="q0")
        q1_t = small.tile([128, TPP], f32, tag="q1")
        s1_t = small.tile([128, TPP], f32, tag="s1")
        s2_t = small.tile([128, TPP], f32, tag="s2")
        p0_t = small.tile([128, TPP], f32, tag="p0")
        p1_t = small.tile([128, TPP], f32, tag="p1")
        z_t = small.tile([128, TPP], f32, tag="z")
        zbuf_t = small.tile([1, N + 1], f32, tag="zbuf")
        r_row_t = small.tile([1, N], f32, tag="r_row")
        tmpa_t = small.tile([128, TPP], f32, tag="tmpa")
        tmpb_t = small.tile([128, TPP], f32, tag="tmpb")
        mu_t = small.tile([128, TPP], f32, tag="mu")
        sinv_t = small.tile([128, TPP], f32, tag="sinv")

        nc.vector.memset(zbuf_t[0:1, 0:1], 0.0)

        stat_tiles = [q0_t, q1_t, s1_t, s2_t]

        for ct in range(4):
            c0 = ct * 512
            st_ps = pst.tile([3, 512], f32, tag="st")
            st2_ps = pst.tile([1, 512], f32, tag="st2")
            for hc in range(8):
                conv_ps = psc.tile([128, 512], f32, tag="conv")
                nmm = 12 * (3 if FP16_CONV else 1)
                mm_i = 0
                for dc in range(4):
                    for k in range(3):
                        woff = (k * 4 + dc) * H + hc * 128
                        xoff = dc * NP2 + c0 + k
                        if FP16_CONV:
                            pairs = [
                                (wth_t[:, woff:woff + 128], xth_t[:, xoff:xoff + 512]),
                                (wtl_t[:, woff:woff + 128], xth_t[:, xoff:xoff + 512]),
                                (wth_t[:, woff:woff + 128], xtl_t[:, xoff:xoff + 512]),
                            ]
                        else:
                            pairs = [
                                (wt_t[:, woff:woff + 128], xt_t[:, xoff:xoff + 512]),
                            ]
                        for lhsT, rhs in pairs:
                            nc.tensor.matmul(
                                conv_ps[:], lhsT, rhs,
                                start=(mm_i == 0), stop=(mm_i == nmm - 1),
                            )
                            mm_i += 1
                # h = relu(conv + bias) on ACT
                h_t = work.tile([128, 512], f32, tag="h")
                nc.scalar.activation(h_t[:], conv_ps[:], AF.Relu,
                                     bias=bias_t[:, hc:hc + 1])
                hsq_t = work.tile([128, 512], f32, tag="hsq")
                nc.vector.tensor_mul(hsq_t[:], h_t[:], h_t[:])
                # stats accumulation: rows 0..2 = (q0,q1,S1) via aug lhsT, row 3 = S2
                nc.tensor.matmul(st_ps[0:3, :], aug_t[:, hc * 4:hc * 4 + 3], h_t[:],
                                 start=(hc == 0), stop=(hc == 7), skip_group_check=True)
                nc.tensor.matmul(st2_ps[0:1, :], aug_t[:, hc * 4 + 2:hc * 4 + 3], hsq_t[:],
                                 start=(hc == 0), stop=(hc == 7), skip_group_check=True)

            # reshape stats [1,512] rows -> [32,16] partition blocks (token-major)
            st_sb = work.tile([3, 512], f32, tag="st_sb")
            st2_sb = work.tile([1, 512], f32, tag="st2_sb")
            nc.vector.tensor_copy(st_sb[:], st_ps[:])
            nc.vector.tensor_copy(st2_sb[:], st2_ps[:])
            for s in range(3):
                nc.sync.dma_start(
                    stat_tiles[s][32 * ct:32 * ct + 32, :], st_sb[s:s + 1, :]
                )
            nc.sync.dma_start(
                stat_tiles[3][32 * ct:32 * ct + 32, :], st2_sb[0:1, :]
            )

            # ---- epilogue for tokens [512ct, 512ct+512) on partitions [32ct, 32ct+32) ----
            pl = slice(32 * ct, 32 * ct + 32)
            mu, sinv, tmpa, tmpb = mu_t[pl, :], sinv_t[pl, :], tmpa_t[pl, :], tmpb_t[pl, :]
            q0, q1, s1, s2 = q0_t[pl, :], q1_t[pl, :], s1_t[pl, :], s2_t[pl, :]
            p0, p1, z = p0_t[pl, :], p1_t[pl, :], z_t[pl, :]
            epi = epi_t[pl.start:pl.stop, :]
            nc.vector.tensor_scalar_mul(mu, s1, 1.0 / H)
            nc.vector.tensor_scalar_mul(tmpa, s2, 1.0 / H)
            nc.vector.tensor_mul(tmpb, mu, mu)
            # var + eps = (tmpa + eps) - tmpb
            nc.vector.scalar_tensor_tensor(
                tmpa, tmpa, 1e-5, tmpb, op0=AL.add, op1=AL.subtract
            )
            nc.scalar.sqrt(tmpa, tmpa)
            nc.vector.reciprocal(sinv, tmpa)
            # p_j = (q_j - mu*G_j) * s + C_j
            nc.vector.tensor_scalar(tmpa, mu, epi[:, 0:1], None, op0=AL.mult)
            nc.vector.tensor_sub(tmpb, q0, tmpa)
            nc.vector.tensor_mul(tmpb, tmpb, sinv)
            nc.vector.tensor_scalar(p0, tmpb, epi[:, 2:3], None, op0=AL.add)
            nc.vector.tensor_scalar(tmpa, mu, epi[:, 1:2], None, op0=AL.mult)
            nc.vector.tensor_sub(tmpb, q1, tmpa)
            nc.vector.tensor_mul(tmpb, tmpb, sinv)
            nc.vector.tensor_scalar(p1, tmpb, epi[:, 3:4], None, op0=AL.add)
            # z = (p1 - p0 <= 0)
            nc.vector.tensor_sub(tmpa, p1, p0)
            nc.vector.tensor_scalar(z, tmpa, 0.0, None, op0=AL.is_le)
            if ct == 0:
                nc.vector.memset(z_t[0:1, 0:1], 0.0)
            # note: z for token N-1 (forced 1 in the reference) feeds zbuf[N],
            # which the scan never reads -- no need to set it.
            # z block -> zbuf[1 + c0 : 1 + c0 + 512]
            nc.sync.dma_start(zbuf_t[0:1, 1 + c0:1 + c0 + 512], z_t[pl, :])
            # exclusive prefix sum for columns [c0, c0+512)
            init = 0.0 if ct == 0 else r_row_t[0:1, c0 - 1:c0]
            nc.vector.tensor_tensor_scan(
                r_row_t[0:1, c0:c0 + 512], zbuf_t[0:1, c0:c0 + 512],
                zbuf_t[0:1, c0:c0 + 512], init, op0=AL.add, op1=AL.bypass,
            )
            # broadcast r to 128 partitions
            rb_ps = psr.tile([128, 512], f32, tag="rb")
            nc.tensor.matmul(rb_ps[:], ones1_t[:], r_row_t[0:1, c0:c0 + 512],
                             start=True, stop=True)
            rb_sb = work.tile([128, 512], f32, tag="rb_sb")
            nc.scalar.copy(rb_sb[:], rb_ps[:])
            # one-hot rows: gm[rt*128 + p, c] = (r[c] == rowbase[p, rt])
            for rt in range(16):
                oh = ohp.tile([128, 512], f32, tag="oh")
                nc.vector.tensor_scalar(
                    oh[:], rb_sb[:], rowbase_t[:, rt:rt + 1], None, op0=AL.is_equal
                )
                nc.sync.dma_start(
                    gm_ext[rt * 128:(rt + 1) * 128, c0:c0 + 512], oh[:]
                )

        # probs out: [2, 128, TPP]
        nc.sync.dma_start(probs_ext[0, :, :], p0_t[:])
        nc.sync.dma_start(probs_ext[1, :, :], p1_t[:])

    _bass_rust.generate_event_semaphores(nc)
    return nc


def _host_prep(x, conv_w, conv_b, ln_g, ln_b, proj_w, proj_b):
    """Build per-core input maps (host-side layout prep)."""
    x = np.asarray(x, np.float32)
    conv_w = np.asarray(conv_w, np.float32)
    conv_b = np.asarray(conv_b, np.float32)
    ln_g = np.asarray(ln_g, np.float32)
    ln_b = np.asarray(ln_b, np.float32)
    proj_w = np.asarray(proj_w, np.float32)
    proj_b = np.asarray(proj_b, np.float32)

    u = ln_g[:, None] * proj_w                      # [H, 2]
    G = ln_g @ proj_w                               # [2]
    C = ln_b @ proj_w + proj_b                      # [2]

    aug = np.zeros((128, 32), np.float32)
    bias = np.zeros((128, 8), np.float32)
    for hc in range(8):
        rows = slice(hc * 128, hc * 128 + 128)
        aug[:, hc * 4 + 0] = u[rows, 0]
        aug[:, hc * 4 + 1] = u[rows, 1]
        aug[:, hc * 4 + 2] = 1.0
        bias[:, hc] = conv_b[rows]
    epi = np.broadcast_to(
        np.array([G[0], G[1], C[0], C[1]], np.float32), (128, 4)
    ).copy()
    rowbase = (np.arange(128, dtype=np.float32)[:, None]
               + 128.0 * np.arange(16, dtype=np.float32)[None, :]).copy()
    ones1 = np.ones((1, 128), np.float32)

    # weights as lhsT [D, H] per (k, dchunk): wt[p, (k*4+dc)*H + hh] = conv_w[hh, dc*128+p, k]
    wt = np.zeros((128, 12 * H), np.float32)
    for k in range(3):
        for dc in range(4):
            blk = conv_w[:, dc * 128:(dc + 1) * 128, k].T  # [128 D, H]
            wt[:, (k * 4 + dc) * H:(k * 4 + dc + 1) * H] = blk

    common = dict(aug=aug, bias=bias, epi=epi, rowbase=rowbase, ones1=ones1)
    if FP16_CONV:
        wth = wt.astype(np.float16)
        wtl = (wt - wth.astype(np.float32)).astype(np.float16)
        common["wth"], common["wtl"] = wth, wtl
    else:
        common["wt"] = wt

    in_maps = []
    for b in range(B):
        xpad = np.zeros((D, NP2), np.float32)
        xpad[:, 1:N + 1] = x[b].T
        xtp = xpad.reshape(4, 128, NP2).transpose(1, 0, 2).reshape(128, 4 * NP2)
        m = dict(common)
        if FP16_CONV:
            xth = xtp.astype(np.float16)
            xtl = (xtp - xth.astype(np.float32)).astype(np.float16)
            m["xth"], m["xtl"] = np.ascontiguousarray(xth), np.ascontiguousarray(xtl)
        else:
            m["xt"] = np.ascontiguousarray(xtp)
        in_maps.append(m)
    return in_maps


def kernel(x, conv_w, conv_b, ln_g, ln_b, proj_w, proj_b, _want_trace=False):
    from concourse.bass_utils import run_bass_kernel_spmd

    key = ("prog", FP16_CONV)
    if key not in _CACHE:
        _CACHE[key] = _build_program()
    nc = _CACHE[key]

    in_maps = _host_prep(x, conv_w, conv_b, ln_g, ln_b, proj_w, proj_b)
    out = run_bass_kernel_spmd(nc, in_maps, list(range(NCORES)), trace=_want_trace)
    res = out.results

    probs = np.zeros((B, N, 2), np.float32)
    gm = np.zeros((B, N, N), np.float32)
    for b in range(B):
        pr = res[b]["probs"]                      # [2, 128, TPP]
        probs[b, :, 0] = pr[0].reshape(N)
        probs[b, :, 1] = pr[1].reshape(N)
        gm[b] = res[b]["gm"]
    if _want_trace:
        return (probs, gm), out
    return (probs, gm)
